# revision 1
# baseline (speedup 1.0000x reference)
"""Trainium2 Bass kernel for nn_AttentionHead (pre-softmax scores variant).

The module returns (q @ k^T * scale) @ v with NO softmax, so the product is
associative:  out = (scale*q) @ (k^T @ v)  with k^T @ v a tiny [64, 64]
matrix.  This removes the [T, T] score matrix entirely: the kernel streams
x once, computes k/v/q projections (3-pass bf16 split-GEMM, fp32-accurate),
a [64, 64] partial S = k^T v, a pairwise AllGather+add across the two cores
holding each batch, and one final tall-skinny matmul.

Sharding: core c <- (batch b = c//2, sequence half h = c%2), 2048 tokens per
core.  Partial S matrices are exchanged within core pairs
[[0,1],[2,3],[4,5],[6,7]].

Host-side marshalling transposes each core's x-chunk so the kernel reads
x^T tiles (contraction dim on partitions) straight from DRAM, and folds the
softmax scale into Wq/bq.
"""

import sys

sys.path.insert(0, "/opt/trn_rl_repo")

import numpy as np

B, T, C, H = 4, 4096, 768, 64
N_CORES = 8
TPC = T // 2  # tokens per core (half a batch's sequence)
CI = C // 128  # 6 contraction chunks
NT = TPC // 512  # 4 moving-dim slices for projections
TI = TPC // 128  # 16 token tiles
SCALE = float(C) ** -0.5

# float32r streams fp32 matmuls at full rate but the PE reduces operand
# precision (~1e-3 relative on hardware); plain float32 runs at 1/4 rate
# but is exact.  Default mode: 3-pass bf16 split-GEMM — x and W are split
# host-side into bf16 hi+lo pairs and the projection runs as
# x_hi@W_hi + x_hi@W_lo + x_lo@W_hi with fp32 PSUM accumulation (exact to
# ~5e-6 relative, 3 cycles/row instead of fp32's 4, same DMA bytes).
USE_F32R = False
USE_BF16_SPLIT = True
PACK_XHL = True  # hi/lo interleaved per chunk in one DRAM tensor (1 MB DMAs)
# walrus --enable-ldw-opt crashes this build; _dedup_ldweights does the same
# elision at the BIR level instead.
ENABLE_LDW_OPT = False

_CACHE = {}


def _patch_ldw_opt():
    """bass_utils hardcodes --enable-ldw-opt=false; consecutive matmuls
    sharing a stationary operand then reload weights every time.  Flip the
    flag so walrus elides redundant LDWEIGHTS."""
    import concourse.bass_utils as bu

    if getattr(bu, "_ldw_opt_patched", False):
        return
    orig = bu.run_command

    def patched(cmd, **kw):
        cmd = [
            "--enable-ldw-opt=true" if c == "--enable-ldw-opt=false" else c
            for c in cmd
        ]
        return orig(cmd, **kw)

    bu.run_command = patched
    bu._ldw_opt_patched = True


def _patch_tile_drain():
    """This walrus build rejects >1 sync wait on TPB_CTRL instructions
    (Drain/NoOp) and the butterfly barrier rides eq-waits on drains.
    Replace the TileContext exit sequence with single-wait nops + plain
    drain + sem-only barriers."""
    import bass_rust as _bass_rust
    import concourse.tile as tile
    from concourse.vector_clock import ScopedClock

    def _drain_and_barrier(self, tick_clock, wait_clock):
        nc = self.nc
        probe = nc.sync.nop(nofuse=True)
        wait_clock.add_sem_waits(
            probe.ins, ScopedClock({None: tick_clock.global_clock})
        )
        waits = list(probe.ins.sync_info.on_wait) if probe.ins.sync_info else []
        updates = list(probe.ins.sync_info.on_update) if probe.ins.sync_info else []
        probe.ins.sync_info = _bass_rust.SyncInfo(
            on_wait=waits[:1], on_update=updates
        )
        for i in range(1, len(waits)):
            extra = nc.sync.nop(nofuse=True)
            extra.ins.sync_info = _bass_rust.SyncInfo(
                on_wait=waits[i : i + 1], on_update=[]
            )
        nc.sync.drain()
        nc.all_engine_barrier(sem_only=True)
        popped = nc._tile_sem_poison_stack.pop()
        assert popped is self._sem_poison
        nc.clear_and_free_semaphores(list(self.sems.allocated().values()))
        nc.all_engine_barrier(sem_only=True)

    tile.TileContext._drain_and_barrier = _drain_and_barrier


def _split_multi_waits(nc):
    """This walrus build allows only ONE sync-wait command per regular
    instruction.  Move extra waits onto dedicated same-engine NOPs placed
    immediately before the instruction (an engine blocks on its own stream,
    so this is semantically identical)."""
    import bass_rust
    import concourse.mybir as mybir

    cnt = 0
    for fn in nc.m.functions:
        for bb in fn.blocks:
            out = []
            for ins in bb.instructions:
                si = ins.sync_info
                if si is not None and si.on_wait and len(si.on_wait) > 1:
                    waits = list(si.on_wait)
                    for w in waits[:-1]:
                        nop = mybir.InstNoOp(name=f"I-waitsplit-{cnt}")
                        cnt += 1
                        nop.engine = ins.engine
                        nop.bass_nofuse = True
                        nop.sync_info = bass_rust.SyncInfo(
                            on_wait=[w], on_update=[]
                        )
                        out.append(nop)
                    ins.sync_info = bass_rust.SyncInfo(
                        on_wait=[waits[-1]], on_update=list(si.on_update or [])
                    )
                out.append(ins)
            bb.instructions = out
    return cnt


def _dedup_ldweights(nc):
    """Tile lowers every non-fp32 matmul into an LDWEIGHTS+MATMUL pair.
    When consecutive PE matmuls share the identical stationary operand the
    reload is redundant (the array already holds it) — delete those
    LDWEIGHTS, reattaching any sync waits to the next instruction."""
    import bass_rust

    def wkey(pap):
        return (str(pap.ap), pap.offset, str(pap.memref))

    removed = 0
    for fn in nc.m.functions:
        for bb in fn.blocks:
            out = []
            last_w = None
            pending_waits = []
            for ins in bb.instructions:
                nm = type(ins).__name__
                if nm == "InstLdweights":
                    k = wkey(ins.ins[0])
                    if last_w == k:
                        if ins.sync_info and ins.sync_info.on_wait:
                            pending_waits.extend(ins.sync_info.on_wait)
                        if ins.sync_info and ins.sync_info.on_update:
                            pending_waits_updates = list(ins.sync_info.on_update)
                            # keep updates by converting into a nop
                            nop = ins  # fallthrough: keep as-is
                            out.append(ins)
                            last_w = k
                            continue
                        removed += 1
                        continue
                    last_w = k
                elif nm == "InstMatmult":
                    if ins.is_transpose:
                        last_w = None  # transpose streams data through the array
                    else:
                        # after execution the array holds this mm's weights
                        # (fp32 matmuls self-load; bf16 ones match their LDW)
                        last_w = wkey(ins.ins[1])
                elif nm in ("InstCompareAndBranch", "InstUnconditionalBranch",
                            "InstCall", "InstDrain"):
                    last_w = None
                if pending_waits and ins.engine is not None:
                    w = list(pending_waits)
                    if ins.sync_info:
                        w = list(ins.sync_info.on_wait) + w
                        upd = list(ins.sync_info.on_update)
                    else:
                        upd = []
                    ins.sync_info = bass_rust.SyncInfo(on_wait=w, on_update=upd)
                    pending_waits = []
                out.append(ins)
            bb.instructions = out
    return removed


def _build_nc(no_collective=False, loop_n=None, internal_x=False, stage=5, walrus_patches=True):
    """loop_n: wrap the whole compute in a For_i hardware loop (timing
    builds only; forces no_collective since collectives cannot sit inside
    control flow).  internal_x: x lives in internal DRAM (uninitialized) so
    timing dispatches skip the 6 MB/core host upload."""
    import concourse.bass as bass
    import concourse.mybir as mybir
    import concourse.tile as tile
    from bass_rust import add_dep_helper

    if loop_n is not None:
        no_collective = True

    if ENABLE_LDW_OPT:
        _patch_ldw_opt()
    _patch_tile_drain()

    f32 = mybir.dt.float32
    bf16 = mybir.dt.bfloat16
    fact = mybir.dt.float32r if USE_F32R else f32

    nc = bass.Bass("TRN2", target_bir_lowering=False, debug=False, num_devices=N_CORES)

    xkind = "Internal" if internal_x else "ExternalInput"
    if USE_BF16_SPLIT:
        if PACK_XHL:
            # hi/lo interleaved per chunk: one contiguous 1 MB DMA per ci.
            xhl = nc.dram_tensor("xhl", [128, CI, 2, TPC], bf16, kind=xkind).ap()
        else:
            xh = nc.dram_tensor("xh", [128, CI, TPC], bf16, kind=xkind).ap()
            xl = nc.dram_tensor("xl", [128, CI, TPC], bf16, kind=xkind).ap()
        wkvh = nc.dram_tensor("wkvh", [128, CI, 128], bf16, kind="ExternalInput").ap()
        wkvl = nc.dram_tensor("wkvl", [128, CI, 128], bf16, kind="ExternalInput").ap()
        wqh = nc.dram_tensor("wqh", [128, CI, H], bf16, kind="ExternalInput").ap()
        wql = nc.dram_tensor("wql", [128, CI, H], bf16, kind="ExternalInput").ap()
    else:
        xt = nc.dram_tensor("xt", [128, CI, TPC], fact, kind=xkind).ap()
        wqk = nc.dram_tensor("wqk", [128, CI, 128], fact, kind="ExternalInput").ap()
        wv = nc.dram_tensor("wv", [128, CI, H], fact, kind="ExternalInput").ap()
    bkv = nc.dram_tensor("bkv", [128, 1], f32, kind="ExternalInput").ap()
    bqp = nc.dram_tensor("bq", [H, 1], f32, kind="ExternalInput").ap()
    ident = nc.dram_tensor("ident", [128, 128], f32, kind="ExternalInput").ap()
    out = nc.dram_tensor("out", [128, 2, 8 * H], f32, kind="ExternalOutput").ap()
    cc_in = nc.dram_tensor("cc_in", [H, H], f32)
    cc_out = nc.dram_tensor("cc_out", [2, H, H], f32)
    RG = [[0, 1], [2, 3], [4, 5], [6, 7]]

    with tile.TileContext(nc) as tc:
        with (
            tc.tile_pool(name="const", bufs=1) as cpool,
            tc.tile_pool(name="data", bufs=1) as dpool,
            tc.tile_pool(name="work", bufs=2) as wpool,
            tc.tile_pool(name="psum", bufs=4, space="PSUM") as ppool,
        ):
            bkv_sb = cpool.tile([128, 1], f32)
            nc.sync.dma_start(out=bkv_sb[:], in_=bkv)
            bq_sb = cpool.tile([H, 1], f32)
            nc.sync.dma_start(out=bq_sb[:], in_=bqp)
            id_sb = cpool.tile([128, 128], f32)
            nc.sync.dma_start(out=id_sb[:], in_=ident)

            if USE_BF16_SPLIT:
                wkvh_sb = cpool.tile([128, CI, 128], bf16)
                nc.sync.dma_start(out=wkvh_sb[:], in_=wkvh)
                wkvl_sb = cpool.tile([128, CI, 128], bf16)
                nc.sync.dma_start(out=wkvl_sb[:], in_=wkvl)
                wqh_sb = cpool.tile([128, CI, H], bf16)
                nc.sync.dma_start(out=wqh_sb[:], in_=wqh)
                wql_sb = cpool.tile([128, CI, H], bf16)
                nc.sync.dma_start(out=wql_sb[:], in_=wql)
            else:
                wqk_sb = cpool.tile([128, CI, 128], fact)
                nc.sync.dma_start(out=wqk_sb[:], in_=wqk)
                wv_sb = cpool.tile([128, CI, H], fact)
                nc.sync.dma_start(out=wv_sb[:], in_=wv)

            def _compute_body(_iv=None):
                if USE_BF16_SPLIT:
                    if PACK_XHL:
                        xhl_sb = dpool.tile([128, CI, 2, TPC], bf16)
                        # first chunk split hi/lo so PE can start after 512 KB
                        nc.sync.dma_start(
                            out=xhl_sb[:, 0, 0, :], in_=xhl[:, 0, 0, :]
                        )
                        nc.sync.dma_start(
                            out=xhl_sb[:, 0, 1, :], in_=xhl[:, 0, 1, :]
                        )
                        for ci in range(1, CI):
                            nc.sync.dma_start(
                                out=xhl_sb[:, ci, :, :], in_=xhl[:, ci, :, :]
                            )
                        xh_sb = xhl_sb[:, :, 0, :]
                        xl_sb = xhl_sb[:, :, 1, :]
                    else:
                        xh_sb = dpool.tile([128, CI, TPC], bf16)
                        xl_sb = dpool.tile([128, CI, TPC], bf16)
                        for ci in range(CI):
                            nc.sync.dma_start(out=xh_sb[:, ci, :], in_=xh[:, ci, :])
                            nc.sync.dma_start(out=xl_sb[:, ci, :], in_=xl[:, ci, :])
                else:
                    xt_sb = dpool.tile([128, CI, TPC], fact)
                    for ci in range(CI):
                        nc.sync.dma_start(out=xt_sb[:, ci, :], in_=xt[:, ci, :])
                if stage < 2:
                    return
                # Projections: kv^T = (Wk | Wv)^T x^T + bias (rows 0..63 k^T,
                # 64..127 v^T); q^T = (scale*Wq)^T x^T + scale*bq.
                kvT = dpool.tile([128, TPC], f32)
                qT = dpool.tile([H, TPC], f32)
                psum_kv = [
                    ppool.tile([128, 512], f32, tag="A", name=f"pkv{nt}")
                    for nt in range(NT)
                ]
                psum_q = [
                    ppool.tile([H, 512], f32, tag="B", name=f"pq{nt}")
                    for nt in range(NT)
                ]
                if USE_BF16_SPLIT:
                    # Pass-major per chunk: consecutive matmuls share the
                    # stationary operand so _dedup_ldweights can elide the
                    # redundant reloads (hi-weights serve both x passes).
                    for ci in range(CI):
                        first = ci == 0
                        last = ci == CI - 1
                        for xs, ws, st, sp in (
                            (xh_sb, wkvh_sb, first, False),
                            (xl_sb, wkvh_sb, False, False),
                            (xh_sb, wkvl_sb, False, last),
                        ):
                            for nt in range(NT):
                                sl = slice(nt * 512, (nt + 1) * 512)
                                nc.tensor.matmul(
                                    psum_kv[nt][:], ws[:, ci, :], xs[:, ci, sl],
                                    start=st, stop=sp,
                                )
                        for xs, ws, st, sp in (
                            (xh_sb, wqh_sb, first, False),
                            (xl_sb, wqh_sb, False, False),
                            (xh_sb, wql_sb, False, last),
                        ):
                            for nt in range(NT):
                                sl = slice(nt * 512, (nt + 1) * 512)
                                nc.tensor.matmul(
                                    psum_q[nt][:], ws[:, ci, :], xs[:, ci, sl],
                                    start=st, stop=sp,
                                )
                else:
                    for ci in range(CI):
                        for nt in range(NT):
                            nc.tensor.matmul(
                                psum_kv[nt][:],
                                wqk_sb[:, ci, :],
                                xt_sb[:, ci, nt * 512 : (nt + 1) * 512],
                                start=(ci == 0),
                                stop=(ci == CI - 1),
                            )
                        for nt in range(NT):
                            nc.tensor.matmul(
                                psum_q[nt][:],
                                wv_sb[:, ci, :],
                                xt_sb[:, ci, nt * 512 : (nt + 1) * 512],
                                start=(ci == 0),
                                stop=(ci == CI - 1),
                            )
                for nt in range(NT):
                    sl = slice(nt * 512, (nt + 1) * 512)
                    nc.vector.tensor_add(
                        out=kvT[:, sl],
                        in0=psum_kv[nt][:],
                        in1=bkv_sb.to_broadcast((128, 512)),
                    )
                    nc.vector.tensor_add(
                        out=qT[:, sl],
                        in0=psum_q[nt][:],
                        in1=bq_sb.to_broadcast((H, 512)),
                    )

                if stage < 3:
                    return
                # Back-transpose kv^T to token-major for the S contraction:
                # one [128,128] transpose per token tile yields both k and v.
                kv_nat = dpool.tile([128, TI, 128], f32)
                for ti in range(TI):
                    tsl = slice(ti * 128, (ti + 1) * 128)
                    pkv_t = ppool.tile([128, 128], f32, tag="A", name="pkvt")
                    nc.tensor.transpose(pkv_t[:], kvT[:, tsl], id_sb[:])
                    nc.vector.tensor_copy(out=kv_nat[:, ti, :], in_=pkv_t[:])

                if stage < 4:
                    return
                # Partial S = k^T v over this core's 2048 tokens.
                psum_s = ppool.tile([H, H], f32, tag="B", name="ps")
                for ti in range(TI):
                    nc.tensor.matmul(
                        psum_s[:],
                        kv_nat[:, ti, 0:H],
                        kv_nat[:, ti, H : 2 * H],
                        start=(ti == 0),
                        stop=(ti == TI - 1),
                    )
                s_sb = wpool.tile([H, H], f32, tag="s")
                nc.vector.tensor_copy(out=s_sb[:], in_=psum_s[:])
                dma_to_cc = nc.sync.dma_start(out=cc_in.ap(), in_=s_sb[:])

                if no_collective:
                    sf_sb = wpool.tile([H, H], f32, tag="sfr")
                    dma_from_cc = nc.sync.dma_start(out=sf_sb[:], in_=cc_in.ap())
                    add_dep_helper(
                        dma_from_cc.ins, dma_to_cc.ins, reason="S readback after write"
                    )
                else:
                    # AllGather (lower latency floor than AllReduce); the pair
                    # sum minus the local partial gives the partner's S without
                    # needing the core's rank.
                    cc = nc.gpsimd.collective_compute(
                        "AllGather",
                        mybir.AluOpType.bypass,
                        replica_groups=RG,
                        ins=[cc_in.ap()],
                        outs=[cc_out.ap()],
                    )
                    add_dep_helper(
                        cc.ins, dma_to_cc.ins, reason="collective waits for S DMA"
                    )
                    sg_sb = wpool.tile([H, 2, H], f32, tag="sg")
                    dma_from_cc = nc.sync.dma_start(
                        out=sg_sb[:], in_=cc_out.ap().rearrange("r p h -> p r h")
                    )
                    add_dep_helper(
                        dma_from_cc.ins, cc.ins, reason="S readback waits for collective"
                    )
                    sf_sb = wpool.tile([H, H], f32, tag="sfr")
                    nc.vector.tensor_add(
                        out=sf_sb[:], in0=sg_sb[:, 0, :], in1=sg_sb[:, 1, :]
                    )

                if stage < 5:
                    return
                # out = (scale*q) @ S_full; 16 ti-outputs pack into 2 psum
                # banks so the epilogue is 2 wide copies + 2 DMAs.
                po_big = [
                    ppool.tile([128, 8 * H], f32, tag="A", name=f"pob{g}")
                    for g in range(2)
                ]
                out_sb = dpool.tile([128, 2, 8 * H], f32)
                for ti in range(TI):
                    tsl = slice(ti * 128, (ti + 1) * 128)
                    osl = slice((ti % 8) * H, (ti % 8 + 1) * H)
                    nc.tensor.matmul(
                        po_big[ti // 8][:, osl], qT[:, tsl], sf_sb[:],
                        start=True, stop=True,
                    )
                for g in range(2):
                    nc.vector.tensor_copy(out=out_sb[:, g, :], in_=po_big[g][:])
                    nc.sync.dma_start(out=out[:, g, :], in_=out_sb[:, g, :])

            if loop_n is not None:
                with tc.For_i(0, loop_n, 1) as _iv:
                    _compute_body(_iv)
            else:
                _compute_body()

    if walrus_patches:
        _dedup_ldweights(nc)
        _split_multi_waits(nc)
    return nc


def _make_runner(**build_kwargs):
    """Build the Bass module once and wrap it in a cached, jitted PJRT
    executable (mirrors bass2jax.run_bass_via_pjrt's multi-core path, but
    reusable across calls so repeat invocations skip trace+compile)."""
    import jax
    from jax.experimental.shard_map import shard_map
    from jax.sharding import Mesh, PartitionSpec

    import concourse.mybir as mybir
    from concourse import bass2jax

    nc = _build_nc(**build_kwargs)
    bass2jax.install_neuronx_cc_hook()

    partition_name = nc.partition_id_tensor.name if nc.partition_id_tensor else None
    in_names, out_names, out_avals, zero_shapes = [], [], [], []
    for alloc in nc.m.functions[0].allocations:
        if not isinstance(alloc, mybir.MemoryLocationSet):
            continue
        name = alloc.memorylocations[0].name
        if alloc.kind == "ExternalInput":
            if name != partition_name:
                in_names.append(name)
        elif alloc.kind == "ExternalOutput":
            out_names.append(name)
            shape = tuple(alloc.tensor_shape)
            dtype = mybir.dt.np(alloc.dtype)
            out_avals.append(jax.core.ShapedArray(shape, dtype))
            zero_shapes.append((shape, dtype))
    n_params = len(in_names)
    in_names_all = list(in_names) + list(out_names)
    if partition_name:
        in_names_all.append(partition_name)

    def _body(*args):
        operands = list(args)
        if partition_name:
            operands.append(bass2jax.partition_id_tensor())
        outs = bass2jax._bass_exec_p.bind(
            *operands,
            out_avals=tuple(out_avals),
            in_names=tuple(in_names_all),
            out_names=tuple(out_names),
            lowering_input_output_aliases=(),
            sim_require_finite=True,
            sim_require_nnan=True,
            nc=nc,
        )
        return tuple(outs)

    devices = jax.devices()[:N_CORES]
    assert len(devices) == N_CORES
    mesh = Mesh(np.asarray(devices), ("core",))
    n_outs = len(out_names)
    sharded = jax.jit(
        shard_map(
            _body,
            mesh=mesh,
            in_specs=(PartitionSpec("core"),) * (n_params + n_outs),
            out_specs=(PartitionSpec("core"),) * n_outs,
            check_rep=False,
        ),
        donate_argnums=tuple(range(n_params, n_params + n_outs)),
        keep_unused=True,
    )
    return {
        "nc": nc,
        "sharded": sharded,
        "in_names": in_names,
        "out_names": out_names,
        "out_avals": out_avals,
        "zero_shapes": zero_shapes,
    }


def _get_runner(**build_kwargs):
    key = ("runner", tuple(sorted(build_kwargs.items())))
    if key not in _CACHE:
        _CACHE[key] = _make_runner(**build_kwargs)
    return _CACHE[key]


def _run(runner, in_maps):
    concat_in = [
        np.concatenate([np.asarray(in_maps[c][nm]) for c in range(N_CORES)], axis=0)
        for nm in runner["in_names"]
    ]
    concat_zeros = [
        np.zeros((N_CORES * shape[0], *shape[1:]), dtype)
        for shape, dtype in runner["zero_shapes"]
    ]
    out_arrs = runner["sharded"](*concat_in, *concat_zeros)
    out_avals = runner["out_avals"]
    return [
        {
            nm: np.asarray(out_arrs[i]).reshape(N_CORES, *out_avals[i].shape)[c]
            for i, nm in enumerate(runner["out_names"])
        }
        for c in range(N_CORES)
    ]


def _bf16_split(a):
    import ml_dtypes

    hi = a.astype(ml_dtypes.bfloat16)
    lo = (a - hi.astype(np.float32)).astype(ml_dtypes.bfloat16)
    return hi, lo


def _prep_inputs(x, Wq, bq, Wk, bk, Wv, bv):
    """Build the 8 per-core input maps (host-side sharding/marshalling)."""
    x = np.asarray(x, dtype=np.float32)
    Wq = np.asarray(Wq, dtype=np.float32)
    Wk = np.asarray(Wk, dtype=np.float32)
    Wv = np.asarray(Wv, dtype=np.float32)
    bq = np.asarray(bq, dtype=np.float32)
    bk = np.asarray(bk, dtype=np.float32)
    bv = np.asarray(bv, dtype=np.float32)

    wkv = np.concatenate([Wk, Wv], axis=1)  # [768, 128]
    wkv = np.ascontiguousarray(wkv.reshape(CI, 128, 128).transpose(1, 0, 2))
    wq_r = np.ascontiguousarray(
        (Wq * SCALE).reshape(CI, 128, H).transpose(1, 0, 2)
    )
    bkv = np.concatenate([bk, bv])[:, None].astype(np.float32)
    bq_r = (bq * SCALE)[:, None].astype(np.float32)
    ident = np.eye(128, dtype=np.float32)

    common = {"bkv": bkv, "bq": bq_r, "ident": ident}
    if USE_BF16_SPLIT:
        wkvh, wkvl = _bf16_split(wkv)
        wqh, wql = _bf16_split(wq_r)
        common.update(
            {"wkvh": wkvh, "wkvl": wkvl, "wqh": wqh, "wql": wql}
        )
    else:
        common.update({"wqk": wkv, "wv": wq_r})

    in_maps = []
    for c in range(N_CORES):
        b, h = divmod(c, 2)
        xc = x[b, h * TPC : (h + 1) * TPC, :]  # [2048, 768]
        xtc = np.ascontiguousarray(
            xc.T.reshape(CI, 128, TPC).transpose(1, 0, 2)
        )  # [128, CI, 2048]
        m = dict(common)
        if USE_BF16_SPLIT:
            hi, lo = _bf16_split(xtc)  # each [128, CI, TPC] bf16
            if PACK_XHL:
                m["xhl"] = np.ascontiguousarray(np.stack([hi, lo], axis=2))
            else:
                m["xh"], m["xl"] = np.ascontiguousarray(hi), np.ascontiguousarray(lo)
        else:
            m["xt"] = xtc
        in_maps.append(m)
    return in_maps


def _assemble(results):
    out = np.empty((B, T, H), dtype=np.float32)
    for c in range(N_CORES):
        b, h = divmod(c, 2)
        oc = results[c]["out"].reshape(128, TI, H)  # partition-major
        out[b, h * TPC : (h + 1) * TPC, :] = oc.transpose(1, 0, 2).reshape(TPC, H)
    return out


def kernel(**inputs):
    runner = _get_runner()
    in_maps = _prep_inputs(**inputs)
    return _assemble(_run(runner, in_maps))



# revision 4
# speedup vs baseline: 19746.7527x; 19746.7527x over previous
"""Trainium2 Bass kernel for nn_AttentionHead (pre-softmax scores variant).

The module returns (q @ k^T * scale) @ v with NO softmax, so the product is
associative:  out = (scale*q) @ (k^T @ v)  with k^T @ v a tiny [64, 64]
matrix.  This removes the [T, T] score matrix entirely: the kernel streams
x once, computes k/v/q projections, a [64, 64] partial S = k^T v, a pairwise
AllGather+add across the two cores holding each batch, and one final
tall-skinny matmul.

Sharding: core c <- (batch b = c//2, sequence half h = c%2), 2048 tokens per
core.  Partial S matrices are exchanged within core pairs
[[0,1],[2,3],[4,5],[6,7]].

The wall-clock cost of a call is dominated by the axon tunnel (~60 MB/s
aggregate), so the host<->device byte count is the primary optimization
axis: x ships as fp16 (24 MB instead of 48), already transposed host-side
(the transpose fuses into the fp32->fp16 cast for free), weights/constants
are content-hashed and kept device-resident across calls, output buffers
are created on-device inside the jit, and the output returns as fp16
(2 MB).  fp16 keeps the end-to-end relative error ~5e-4.
"""

import hashlib
import sys

sys.path.insert(0, "/opt/trn_rl_repo")

import numpy as np

B, T, C, H = 4, 4096, 768, 64
N_CORES = 8
TPC = T // 2  # tokens per core (half a batch's sequence)
CI = C // 128  # 6 contraction chunks
NT = TPC // 512  # 4 moving-dim slices for projections
TI = TPC // 128  # 16 token tiles
SCALE = float(C) ** -0.5

_CACHE = {}


def _patch_tile_drain():
    """This walrus build rejects >1 sync wait on TPB_CTRL instructions
    (Drain/NoOp) and the butterfly barrier rides eq-waits on drains.
    Replace the TileContext exit sequence with single-wait nops + plain
    drain + sem-only barriers."""
    import bass_rust as _bass_rust
    import concourse.tile as tile
    from concourse.vector_clock import ScopedClock

    def _drain_and_barrier(self, tick_clock, wait_clock):
        nc = self.nc
        probe = nc.sync.nop(nofuse=True)
        wait_clock.add_sem_waits(
            probe.ins, ScopedClock({None: tick_clock.global_clock})
        )
        waits = list(probe.ins.sync_info.on_wait) if probe.ins.sync_info else []
        updates = list(probe.ins.sync_info.on_update) if probe.ins.sync_info else []
        probe.ins.sync_info = _bass_rust.SyncInfo(
            on_wait=waits[:1], on_update=updates
        )
        for i in range(1, len(waits)):
            extra = nc.sync.nop(nofuse=True)
            extra.ins.sync_info = _bass_rust.SyncInfo(
                on_wait=waits[i : i + 1], on_update=[]
            )
        nc.sync.drain()
        nc.all_engine_barrier(sem_only=True)
        popped = nc._tile_sem_poison_stack.pop()
        assert popped is self._sem_poison
        nc.clear_and_free_semaphores(list(self.sems.allocated().values()))
        nc.all_engine_barrier(sem_only=True)

    tile.TileContext._drain_and_barrier = _drain_and_barrier


def _split_multi_waits(nc):
    """This walrus build allows only ONE sync-wait command per regular
    instruction.  Move extra waits onto dedicated same-engine NOPs placed
    immediately before the instruction (an engine blocks on its own stream,
    so this is semantically identical)."""
    import bass_rust
    import concourse.mybir as mybir

    cnt = 0
    for fn in nc.m.functions:
        for bb in fn.blocks:
            out = []
            for ins in bb.instructions:
                si = ins.sync_info
                if si is not None and si.on_wait and len(si.on_wait) > 1:
                    waits = list(si.on_wait)
                    for w in waits[:-1]:
                        nop = mybir.InstNoOp(name=f"I-waitsplit-{cnt}")
                        cnt += 1
                        nop.engine = ins.engine
                        nop.bass_nofuse = True
                        nop.sync_info = bass_rust.SyncInfo(
                            on_wait=[w], on_update=[]
                        )
                        out.append(nop)
                    ins.sync_info = bass_rust.SyncInfo(
                        on_wait=[waits[-1]], on_update=list(si.on_update or [])
                    )
                out.append(ins)
            bb.instructions = out
    return cnt


def _dedup_ldweights(nc):
    """Tile lowers every non-fp32 matmul into an LDWEIGHTS+MATMUL pair.
    When consecutive PE matmuls share the identical stationary operand the
    reload is redundant (the array already holds it) — delete those
    LDWEIGHTS, reattaching any sync waits to the next instruction."""
    import bass_rust

    def wkey(pap):
        return (str(pap.ap), pap.offset, str(pap.memref))

    removed = 0
    for fn in nc.m.functions:
        for bb in fn.blocks:
            out = []
            last_w = None
            pending_waits = []
            for ins in bb.instructions:
                nm = type(ins).__name__
                if nm == "InstLdweights":
                    k = wkey(ins.ins[0])
                    if last_w == k:
                        if ins.sync_info and ins.sync_info.on_wait:
                            pending_waits.extend(ins.sync_info.on_wait)
                        if ins.sync_info and ins.sync_info.on_update:
                            out.append(ins)
                            last_w = k
                            continue
                        removed += 1
                        continue
                    last_w = k
                elif nm == "InstMatmult":
                    if ins.is_transpose:
                        last_w = None  # transpose streams data through the array
                    else:
                        last_w = wkey(ins.ins[1])
                elif nm in ("InstCompareAndBranch", "InstUnconditionalBranch",
                            "InstCall", "InstDrain"):
                    last_w = None
                if pending_waits and ins.engine is not None:
                    w = list(pending_waits)
                    if ins.sync_info:
                        w = list(ins.sync_info.on_wait) + w
                        upd = list(ins.sync_info.on_update)
                    else:
                        upd = []
                    ins.sync_info = bass_rust.SyncInfo(on_wait=w, on_update=upd)
                    pending_waits = []
                out.append(ins)
            bb.instructions = out
    return removed


def _build_nc(no_collective=False, walrus_patches=True):
    import concourse.bass as bass
    import concourse.mybir as mybir
    import concourse.tile as tile
    from bass_rust import add_dep_helper

    _patch_tile_drain()

    f32 = mybir.dt.float32
    f16 = mybir.dt.float16

    nc = bass.Bass("TRN2", target_bir_lowering=False, debug=False,
                   num_devices=N_CORES)

    xt = nc.dram_tensor("xt", [128, CI, TPC], f16, kind="ExternalInput").ap()
    wkv = nc.dram_tensor("wkv", [128, CI, 128], f16, kind="ExternalInput").ap()
    wq = nc.dram_tensor("wq", [128, CI, H], f16, kind="ExternalInput").ap()
    bkv = nc.dram_tensor("bkv", [128, 1], f32, kind="ExternalInput").ap()
    bqp = nc.dram_tensor("bq", [H, 1], f32, kind="ExternalInput").ap()
    ident = nc.dram_tensor("ident", [128, 128], f16, kind="ExternalInput").ap()
    out = nc.dram_tensor("out", [128, 2, 8 * H], f16, kind="ExternalOutput").ap()
    cc_in = nc.dram_tensor("cc_in", [H, H], f32)
    cc_out = nc.dram_tensor("cc_out", [2, H, H], f32)
    RG = [[0, 1], [2, 3], [4, 5], [6, 7]]

    with tile.TileContext(nc) as tc:
        with (
            tc.tile_pool(name="const", bufs=1) as cpool,
            tc.tile_pool(name="data", bufs=1) as dpool,
            tc.tile_pool(name="work", bufs=2) as wpool,
            tc.tile_pool(name="psum", bufs=4, space="PSUM") as ppool,
        ):
            bkv_sb = cpool.tile([128, 1], f32)
            nc.sync.dma_start(out=bkv_sb[:], in_=bkv)
            bq_sb = cpool.tile([H, 1], f32)
            nc.sync.dma_start(out=bq_sb[:], in_=bqp)
            id_sb = cpool.tile([128, 128], f16)
            nc.sync.dma_start(out=id_sb[:], in_=ident)
            wkv_sb = cpool.tile([128, CI, 128], f16)
            nc.sync.dma_start(out=wkv_sb[:], in_=wkv)
            wq_sb = cpool.tile([128, CI, H], f16)
            nc.sync.dma_start(out=wq_sb[:], in_=wq)

            # x^T arrives pre-transposed from the host; chunk-major DMAs so
            # the PE can start on chunk 0 after 512 KB.
            xt_sb = dpool.tile([128, CI, TPC], f16)
            for ci in range(CI):
                nc.sync.dma_start(out=xt_sb[:, ci, :], in_=xt[:, ci, :])

            # kv^T = (Wk | Wv)^T x^T + bias (rows 0..63 k^T, 64..127 v^T).
            # kv first so the S collective launches before the q work.
            kvT = dpool.tile([128, TPC], f16)
            qT = dpool.tile([H, TPC], f16)
            psum_kv = [
                ppool.tile([128, 512], f32, tag="A", name=f"pkv{nt}")
                for nt in range(NT)
            ]
            psum_q = [
                ppool.tile([H, 512], f32, tag="B", name=f"pq{nt}")
                for nt in range(NT)
            ]
            for ci in range(CI):
                for nt in range(NT):
                    sl = slice(nt * 512, (nt + 1) * 512)
                    nc.tensor.matmul(
                        psum_kv[nt][:], wkv_sb[:, ci, :], xt_sb[:, ci, sl],
                        start=(ci == 0), stop=(ci == CI - 1),
                    )
            for nt in range(NT):
                sl = slice(nt * 512, (nt + 1) * 512)
                nc.vector.tensor_add(
                    out=kvT[:, sl],
                    in0=psum_kv[nt][:],
                    in1=bkv_sb.to_broadcast((128, 512)),
                )

            # Back-transpose kv^T to token-major for the S contraction.
            kv_nat = dpool.tile([128, TI, 128], f16)
            for ti in range(TI):
                tsl = slice(ti * 128, (ti + 1) * 128)
                pkv_t = ppool.tile([128, 128], f16, tag="A", name="pkvt")
                nc.tensor.transpose(pkv_t[:], kvT[:, tsl], id_sb[:])
                nc.vector.tensor_copy(out=kv_nat[:, ti, :], in_=pkv_t[:])

            # Partial S = k^T v over this core's 2048 tokens.
            psum_s = ppool.tile([H, H], f32, tag="B", name="ps")
            for ti in range(TI):
                nc.tensor.matmul(
                    psum_s[:],
                    kv_nat[:, ti, 0:H],
                    kv_nat[:, ti, H : 2 * H],
                    start=(ti == 0),
                    stop=(ti == TI - 1),
                )
            s_sb = wpool.tile([H, H], f32, tag="s")
            nc.vector.tensor_copy(out=s_sb[:], in_=psum_s[:])
            dma_to_cc = nc.sync.dma_start(out=cc_in.ap(), in_=s_sb[:])

            if no_collective:
                sf_sb = wpool.tile([H, H], f32, tag="sfr")
                dma_from_cc = nc.sync.dma_start(out=sf_sb[:], in_=cc_in.ap())
                add_dep_helper(
                    dma_from_cc.ins, dma_to_cc.ins, reason="S readback after write"
                )
            else:
                # AllGather (lower latency floor than AllReduce); the pair
                # sum minus the local partial gives the partner's S without
                # needing the core's rank.
                cc = nc.gpsimd.collective_compute(
                    "AllGather",
                    mybir.AluOpType.bypass,
                    replica_groups=RG,
                    ins=[cc_in.ap()],
                    outs=[cc_out.ap()],
                )
                add_dep_helper(
                    cc.ins, dma_to_cc.ins, reason="collective waits for S DMA"
                )
                sg_sb = wpool.tile([H, 2, H], f32, tag="sg")
                dma_from_cc = nc.sync.dma_start(
                    out=sg_sb[:], in_=cc_out.ap().rearrange("r p h -> p r h")
                )
                add_dep_helper(
                    dma_from_cc.ins, cc.ins, reason="S readback waits for collective"
                )
                sf_sb = wpool.tile([H, H], f32, tag="sfr")
                nc.vector.tensor_add(
                    out=sf_sb[:], in0=sg_sb[:, 0, :], in1=sg_sb[:, 1, :]
                )

            # q^T = (scale*Wq)^T x^T + scale*bq, overlapping the collective.
            for ci in range(CI):
                for nt in range(NT):
                    sl = slice(nt * 512, (nt + 1) * 512)
                    nc.tensor.matmul(
                        psum_q[nt][:], wq_sb[:, ci, :], xt_sb[:, ci, sl],
                        start=(ci == 0), stop=(ci == CI - 1),
                    )
            for nt in range(NT):
                sl = slice(nt * 512, (nt + 1) * 512)
                nc.vector.tensor_add(
                    out=qT[:, sl],
                    in0=psum_q[nt][:],
                    in1=bq_sb.to_broadcast((H, 512)),
                )

            sf16 = wpool.tile([H, H], f16, tag="sf16")
            nc.vector.tensor_copy(out=sf16[:], in_=sf_sb[:])

            # out = (scale*q) @ S_full; 16 ti-outputs pack into 2 psum
            # banks so the epilogue is 2 wide copies + 2 DMAs.
            po_big = [
                ppool.tile([128, 8 * H], f32, tag="A", name=f"pob{g}")
                for g in range(2)
            ]
            out_sb = dpool.tile([128, 2, 8 * H], f16)
            for ti in range(TI):
                tsl = slice(ti * 128, (ti + 1) * 128)
                osl = slice((ti % 8) * H, (ti % 8 + 1) * H)
                nc.tensor.matmul(
                    po_big[ti // 8][:, osl], qT[:, tsl], sf16[:],
                    start=True, stop=True,
                )
            for g in range(2):
                nc.vector.tensor_copy(out=out_sb[:, g, :], in_=po_big[g][:])
                nc.sync.dma_start(out=out[:, g, :], in_=out_sb[:, g, :])

    if walrus_patches:
        _dedup_ldweights(nc)
        _split_multi_waits(nc)
    return nc


def _make_runner(**build_kwargs):
    """Build the Bass module once and wrap it in a cached, jitted PJRT
    executable.  Output buffers are created on-device inside the jit (no
    host->device zero upload)."""
    import jax
    import jax.numpy as jnp
    from jax.experimental.shard_map import shard_map
    from jax.sharding import Mesh, NamedSharding, PartitionSpec

    import concourse.mybir as mybir
    from concourse import bass2jax

    nc = _build_nc(**build_kwargs)
    bass2jax.install_neuronx_cc_hook()

    partition_name = nc.partition_id_tensor.name if nc.partition_id_tensor else None
    in_names, out_names, out_avals, zero_shapes = [], [], [], []
    for alloc in nc.m.functions[0].allocations:
        if not isinstance(alloc, mybir.MemoryLocationSet):
            continue
        name = alloc.memorylocations[0].name
        if alloc.kind == "ExternalInput":
            if name != partition_name:
                in_names.append(name)
        elif alloc.kind == "ExternalOutput":
            out_names.append(name)
            shape = tuple(alloc.tensor_shape)
            dtype = mybir.dt.np(alloc.dtype)
            out_avals.append(jax.core.ShapedArray(shape, dtype))
            zero_shapes.append((shape, dtype))
    n_params = len(in_names)
    in_names_all = list(in_names) + list(out_names)
    if partition_name:
        in_names_all.append(partition_name)

    def _body(*args):
        operands = list(args)
        if partition_name:
            operands.append(bass2jax.partition_id_tensor())
        outs = bass2jax._bass_exec_p.bind(
            *operands,
            out_avals=tuple(out_avals),
            in_names=tuple(in_names_all),
            out_names=tuple(out_names),
            lowering_input_output_aliases=(),
            sim_require_finite=True,
            sim_require_nnan=True,
            nc=nc,
        )
        return tuple(outs)

    devices = jax.devices()[:N_CORES]
    assert len(devices) == N_CORES
    mesh = Mesh(np.asarray(devices), ("core",))
    sharding = NamedSharding(mesh, PartitionSpec("core"))
    n_outs = len(out_names)
    sharded = jax.jit(
        shard_map(
            _body,
            mesh=mesh,
            in_specs=(PartitionSpec("core"),) * (n_params + n_outs),
            out_specs=(PartitionSpec("core"),) * n_outs,
            check_rep=False,
        ),
        donate_argnums=tuple(range(n_params, n_params + n_outs)),
        keep_unused=True,
    )
    return {
        "nc": nc,
        "sharded": sharded,
        "sharding": sharding,
        "in_names": in_names,
        "out_names": out_names,
        "out_avals": out_avals,
        "zero_shapes": zero_shapes,
        "out_bufs": None,  # device-resident recycled output buffers
    }


def _get_runner(**build_kwargs):
    key = ("runner", tuple(sorted(build_kwargs.items())))
    if key not in _CACHE:
        _CACHE[key] = _make_runner(**build_kwargs)
    return _CACHE[key]


def _prep_consts(runner, Wq, bq, Wk, bk, Wv, bv):
    """Marshal + device_put the weight/constant tensors, cached by content
    hash so repeat calls with unchanged weights skip the upload."""
    import jax

    h = hashlib.blake2b(digest_size=16)
    for a in (Wq, bq, Wk, bk, Wv, bv):
        h.update(np.ascontiguousarray(a).tobytes())
    key = ("consts", h.hexdigest())
    if key in _CACHE:
        return _CACHE[key]

    Wq = np.asarray(Wq, np.float32)
    Wk = np.asarray(Wk, np.float32)
    Wv = np.asarray(Wv, np.float32)
    bq_ = np.asarray(bq, np.float32)
    bk_ = np.asarray(bk, np.float32)
    bv_ = np.asarray(bv, np.float32)

    wkv = np.concatenate([Wk, Wv], axis=1)  # [768, 128]
    wkv_r = wkv.reshape(CI, 128, 128).transpose(1, 0, 2).astype(np.float16)
    wq_r = (Wq * SCALE).reshape(CI, 128, H).transpose(1, 0, 2).astype(np.float16)
    bkv = np.concatenate([bk_, bv_])[:, None].astype(np.float32)
    bq_r = (bq_ * SCALE)[:, None].astype(np.float32)
    id16 = np.eye(128, dtype=np.float16)

    per_core = {
        "wkv": np.ascontiguousarray(wkv_r),
        "wq": np.ascontiguousarray(wq_r),
        "bkv": bkv,
        "bq": bq_r,
        "ident": id16,
    }
    devs = {
        nm: jax.device_put(
            np.concatenate([per_core[nm]] * N_CORES, axis=0), runner["sharding"]
        )
        for nm in per_core
    }
    _CACHE[key] = devs
    return devs


def _prep_x(x):
    """x [B, T, C] fp32 -> per-core x^T fp16, concatenated on axis 0 for the
    sharded call.  The transpose rides inside the fp32->fp16 cast for free."""
    x = np.asarray(x)
    xt = (
        x.reshape(B, 2, TPC, CI, 128)
        .transpose(0, 1, 4, 3, 2)
        .astype(np.float16)
    )
    return xt.reshape(N_CORES * 128, CI, TPC)


def _assemble(out_np):
    """[N_CORES*128, 2, 8H] fp16 -> [B, T, H] fp32."""
    oc = out_np.reshape(B, 2, 128, 2, 8, H)
    # token within half = g*1024 + k*128 + p  (g psum group, k tile-in-group)
    full = oc.transpose(0, 1, 3, 4, 2, 5).astype(np.float32)
    return full.reshape(B, T, H)


def kernel(**inputs):
    import jax

    runner = _get_runner()
    consts = _prep_consts(
        runner,
        inputs["Wq"], inputs["bq"],
        inputs["Wk"], inputs["bk"],
        inputs["Wv"], inputs["bv"],
    )
    xt = _prep_x(inputs["x"])
    x_dev = jax.device_put(xt, runner["sharding"])
    args = [x_dev if nm == "xt" else consts[nm] for nm in runner["in_names"]]
    # The kernel fully overwrites its output tensors, so their incoming
    # contents are irrelevant: recycle the previous call's device-resident
    # outputs as this call's donated buffers (first call uploads zeros once).
    out_bufs = runner["out_bufs"]
    if out_bufs is None:
        out_bufs = [
            jax.device_put(
                np.zeros((N_CORES * s[0], *s[1:]), d), runner["sharding"]
            )
            for s, d in runner["zero_shapes"]
        ]
    outs = runner["sharded"](*args, *out_bufs)
    out_np = np.asarray(outs[0])
    runner["out_bufs"] = list(outs)
    return _assemble(out_np)
